# revision 12
# baseline (speedup 1.0000x reference)
"""BiMamba block kernel for Trainium2, 8 NeuronCores.

Sharding: 8 cores = 4 batches x 2 directions (fwd/bwd). Each core runs the
full mamba2 for one (batch, direction) with the sequence pre-flipped on the
host for bwd cores, and computes its half of the final projection. The host
combines: out[b] = x[b] + proj_b + part_fwd[b] + flip(part_bwd[b]).

Per-core pipeline (l=2048 processed in 2 halves of 1024):
  in_proj (f32r matmuls, x-rmsnorm folded in at psum evacuation) -> causal
  depthwise conv + silu (feature-major) -> chunked SSD selective scan
  (Q=128 chunks; matmul-based intra-chunk attention with difference-form
  decay, heads batched where operands are shared) -> gate + rmsnorm ->
  out_proj -> final proj half (bf16 matmuls).
"""
import contextlib

import numpy as np
import ml_dtypes

import bass_rust
import concourse.bass as bass
import concourse.mybir as mybir
import concourse.tile as tile
from concourse.alu_op_type import AluOpType
from concourse.bass_utils import run_bass_kernel_spmd
from concourse.vector_clock import ScopedClock

F32 = mybir.dt.float32
F32R = mybir.dt.float32r
BF16 = mybir.dt.bfloat16
F16 = mybir.dt.float16
AF = mybir.ActivationFunctionType

D_MODEL = 1024
D_STATE = 128
D_CONV = 4
HEADDIM = 64
D_INNER = 2048
NHEADS = 32
CONV_DIM = D_INNER + 2 * D_STATE          # 2304
L = 2048
Q = 128                                    # scan chunk
HALF = 1024                                # seq processed per pass
NCH = HALF // Q                            # chunks per half
NXT = D_INNER // 128                       # 16 x-part feature tiles
NBCT = CONV_DIM // 128                     # 18 conv feature tiles


# ---------------------------------------------------------------------------
# Tile/walrus compatibility patches: this toolchain's walrus rejects >1 sync
# wait per instruction; split extra waits onto same-engine nops.
# ---------------------------------------------------------------------------
def _split_drain_and_barrier(self, tick_clock, wait_clock):
    nc = self.nc
    drain_inst = nc.sync.drain()
    wait_clock.add_sem_waits(
        drain_inst.ins, ScopedClock({None: tick_clock.global_clock})
    )
    si = drain_inst.ins.sync_info
    waits = list(si.on_wait or []) if si is not None else []
    if len(waits) > 1:
        si.on_wait = waits[:1]
        drain_inst.ins.sync_info = si
        for i in range(1, len(waits)):
            n2 = nc.sync.nop()
            n2.ins.sync_info = bass_rust.SyncInfo(on_wait=[waits[i]], on_update=[])
    nc.all_engine_barrier()
    assert self.sems is not None
    popped = nc._tile_sem_poison_stack.pop()
    assert popped is self._sem_poison
    nc.clear_and_free_semaphores(list(self.sems.allocated().values()))
    nc.all_engine_barrier()


tile.TileContext._drain_and_barrier = _split_drain_and_barrier


def split_multi_waits(nc, maxw=1):
    cnt = 0
    for f in nc.m.functions:
        for bb in f.blocks:
            insts = bb.instructions
            i = 0
            while i < len(insts):
                inst = insts[i]
                si = inst.sync_info
                waits = list(si.on_wait) if (si is not None and si.on_wait) else []
                if len(waits) > maxw:
                    si.on_wait = waits[:maxw]
                    inst.sync_info = si
                    for j in range(maxw, len(waits), maxw):
                        n = mybir.InstNoOp(name=f"I-wsplit-{cnt}")
                        cnt += 1
                        n.engine = inst.engine
                        n.sync_info = bass_rust.SyncInfo(
                            on_wait=waits[j : j + maxw], on_update=[]
                        )
                        insts.insert(i, n)
                        i += 1
                i += 1
    return cnt


# ---------------------------------------------------------------------------
# Device program (identical on all 8 cores; data differs per core)
# ---------------------------------------------------------------------------
def build_program():
    nc = bass.Bass(target_bir_lowering=False)

    xT = nc.dram_tensor("xT", [D_MODEL, L], F32, kind="ExternalInput")
    wxbc = nc.dram_tensor("wxbc", [D_MODEL, CONV_DIM + NHEADS], F32,
                          kind="ExternalInput")
    wz = nc.dram_tensor("wz", [D_MODEL, D_INNER], F32, kind="ExternalInput")
    wout = nc.dram_tensor("wout", [D_INNER, D_MODEL], F16, kind="ExternalInput")
    wp = nc.dram_tensor("wp", [D_MODEL, D_MODEL], F16, kind="ExternalInput")
    convw = nc.dram_tensor("convw", [CONV_DIM, D_CONV], F32, kind="ExternalInput")
    convb = nc.dram_tensor("convb", [CONV_DIM], F32, kind="ExternalInput")
    dtb = nc.dram_tensor("dtb", [NHEADS], F32, kind="ExternalInput")
    alog = nc.dram_tensor("alog", [NHEADS], F32, kind="ExternalInput")
    dvec = nc.dram_tensor("dvec", [NHEADS], F32, kind="ExternalInput")
    tri = nc.dram_tensor("tri", [Q, Q], F16, kind="ExternalInput")
    identw = nc.dram_tensor("identw", [128, 128], F32, kind="ExternalInput")
    o = nc.dram_tensor("o", [L, D_MODEL], F32, kind="ExternalOutput")
    dbg_dt = nc.dram_tensor("dbg_dt", [NHEADS, HALF], F32, kind="ExternalOutput")
    dbg_cum = nc.dram_tensor("dbg_cum", [NHEADS, Q], F32, kind="ExternalOutput")
    dbg_x0 = nc.dram_tensor("dbg_x0", [128, Q], F32, kind="ExternalOutput")
    dbg_B = nc.dram_tensor("dbg_B", [128, Q], F32, kind="ExternalOutput")
    dbg_e = nc.dram_tensor("dbg_e", [128, 512], F32, kind="ExternalOutput")
    dbg_m = nc.dram_tensor("dbg_m", [128, 512], F32, kind="ExternalOutput")
    dbg_xtok = nc.dram_tensor("dbg_xtok", [128, 512], F32, kind="ExternalOutput")
    dbg_t3 = nc.dram_tensor("dbg_t3", [Q, 512], F32, kind="ExternalOutput")
    dbg_yn = nc.dram_tensor("dbg_yn", [Q, 512], F32, kind="ExternalOutput")

    z_dram = nc.dram_tensor("z_dram", [L, D_INNER], F16)
    rstd_dram = nc.dram_tensor("rstd_dram", [2, HALF], F32)
    cumrow_dram = nc.dram_tensor("cumrow_dram", [16, NHEADS * Q], F32)
    cend_dram = nc.dram_tensor("cend_dram", [16, NHEADS], F32)

    with tile.TileContext(nc) as tc:
        _build_body(nc, tc, xT, wxbc, wz, wout, wp, convw, convb, dtb, alog,
                    dvec, tri, identw, o, z_dram, rstd_dram, cumrow_dram,
                    cend_dram, DBG=dict(dt=dbg_dt, cum=dbg_cum, x0=dbg_x0,
                                        B=dbg_B, e=dbg_e, m=dbg_m,
                                        xtok=dbg_xtok, t3=dbg_t3, yn=dbg_yn))
    split_multi_waits(nc)
    return nc


def _build_body(nc, tc, xT, wxbc, wz, wout, wp, convw, convb, dtb, alog,
                dvec, tri, identw, o, z_dram, rstd_dram, cumrow_dram,
                cend_dram, DBG=None):
    with contextlib.ExitStack() as ctx:
        consts = ctx.enter_context(tc.tile_pool(name="consts", bufs=1))
        persist = ctx.enter_context(tc.tile_pool(name="persist", bufs=1))
        wpool = ctx.enter_context(tc.tile_pool(name="wpool", bufs=4))

        # ---- constants
        ident = consts.tile([128, 128], F32)
        nc.sync.dma_start(out=ident, in_=identw[:, :])
        ident_bf = consts.tile([128, 128], F16)
        nc.vector.tensor_copy(out=ident_bf, in_=ident)
        tri_bf = consts.tile([Q, Q], F16)
        nc.sync.dma_start(out=tri_bf, in_=tri[:, :])
        ones_f16 = consts.tile([128, 1], F16)
        nc.vector.memset(ones_f16, 1.0)
        ones32 = consts.tile([NHEADS, Q], F32)
        nc.vector.memset(ones32, 1.0)
        eps1 = consts.tile([1, 1], F32)
        nc.vector.memset(eps1, 1e-4)
        eps2 = consts.tile([128, 1], F32)
        nc.vector.memset(eps2, 1e-5)

        dtb_sb = consts.tile([NHEADS, 1], F32)
        nc.sync.dma_start(out=dtb_sb, in_=dtb[:, None])
        alog_sb = consts.tile([NHEADS, 1], F32)
        nc.sync.dma_start(out=alog_sb, in_=alog[:, None])
        lam = consts.tile([NHEADS, 1], F32)
        nc.scalar.activation(out=lam, in_=alog_sb, func=AF.Exp)
        lamneg = consts.tile([NHEADS, 1], F32)
        nc.vector.tensor_scalar(out=lamneg, in0=lam, scalar1=-1.0, scalar2=None,
                                op0=AluOpType.mult)
        d_bc = consts.tile([128, NHEADS], F32)
        nc.sync.dma_start(out=d_bc, in_=dvec[None, :].broadcast_to([128, NHEADS]))

        convb_sb = consts.tile([128, NBCT], F32)
        nc.sync.dma_start(out=convb_sb, in_=convb.rearrange("(t p) -> p t", p=128))
        convw_sb = consts.tile([128, NBCT, D_CONV], F32)
        nc.sync.dma_start(out=convw_sb,
                          in_=convw.rearrange("(t p) k -> p t k", p=128))

        # final-proj weights resident (bf16, 2 MB)
        wp_sb = [persist.tile([128, D_MODEL], F16, tag=f"wp{k}", name=f"wp{k}") for k in range(8)]
        for k in range(8):
            nc.sync.dma_start(out=wp_sb[k], in_=wp[k * 128:(k + 1) * 128, :])

        # cross-half state
        S_bf = [persist.tile([D_STATE, D_INNER], F16, tag=f"S{i}", name=f"S{i}") for i in range(2)]
        nc.vector.memset(S_bf[0], 0.0)
        tails = persist.tile([128, NBCT, D_CONV - 1], F32)
        nc.vector.memset(tails, 0.0)

        # per-half feature-major activations (overwritten each half)
        xcv = [persist.tile([128, HALF], F16, tag=f"xcv{t}", name=f"xcv{t}") for t in range(NXT)]
        Bfm = persist.tile([128, HALF], F16, tag="Bfm")
        Cfm = persist.tile([128, HALF], F16, tag="Cfm")
        dtfm = persist.tile([NHEADS, HALF], F32, tag="dtfm")
        logda = persist.tile([NHEADS, HALF], F32, tag="logda")

        sidx = 0  # S ping-pong index

        for half in range(2):
            l0 = half * HALF

            # ================= phase 1: in_proj + conv + z =================
            with tc.tile_pool(name=f"xin{half}", bufs=1) as xin, \
                 tc.tile_pool(name=f"work{half}", bufs=2) as work, \
                 tc.tile_pool(name=f"psin{half}", bufs=3, space="PSUM") as ps_in, \
                 tc.tile_pool(name=f"pssm{half}", bufs=1, space="PSUM") as ps_sm1:
                xt = [xin.tile([128, HALF], F32R, tag=f"xt{k}", name=f"xt{half}_{k}") for k in range(8)]
                for k in range(8):
                    nc.sync.dma_start(
                        out=xt[k],
                        in_=xT[k * 128:(k + 1) * 128, l0:l0 + HALF].bitcast(F32R))

                # rmsnorm(x): column sums of squares via PE ones-reduce
                ssq_ps = ps_sm1.tile([1, HALF], F32, tag="sm1")
                for k in range(8):
                    sq = work.tile([128, HALF], F16, tag="sq")
                    nc.scalar.activation(out=sq, in_=xt[k].bitcast(F32),
                                         func=AF.Square)
                    for lc in range(2):
                        nc.tensor.matmul(out=ssq_ps[:, lc * 512:(lc + 1) * 512],
                                         lhsT=ones_f16,
                                         rhs=sq[:, lc * 512:(lc + 1) * 512],
                                         start=(k == 0), stop=(k == 7))
                lnv = work.tile([1, HALF], F32, tag="lnv")
                nc.scalar.activation(out=lnv, in_=ssq_ps, func=AF.Ln,
                                     bias=eps1, scale=1.0 / D_MODEL)
                rstd_row = work.tile([1, HALF], F32, tag="rstdrow")
                nc.scalar.activation(out=rstd_row, in_=lnv, func=AF.Exp,
                                     scale=-0.5)
                nc.sync.dma_start(out=rstd_dram[half:half + 1, :], in_=rstd_row)
                rstdB = xin.tile([128, HALF], F32, tag="rstdB")
                nc.sync.dma_start(out=rstdB,
                                  in_=rstd_dram[half:half + 1, :].broadcast_to([128, HALF]))
                rstd_tok = work.tile([128, NCH], F32, tag="rstdtok")
                nc.sync.dma_start(
                    out=rstd_tok,
                    in_=rstd_dram[half:half + 1, :].rearrange("o (t p) -> (o p) t", p=128))

                # in_proj xBC (feature-major) + conv + silu
                for ft in range(NBCT):
                    px = ps_in.tile([128, HALF], F32, tag="pin")
                    for lc in range(2):
                        for k in range(8):
                            wblk = wpool.tile([128, 128], F32R, tag="wblk")
                            nc.sync.dma_start(
                                out=wblk,
                                in_=wxbc[k * 128:(k + 1) * 128,
                                         ft * 128:(ft + 1) * 128].bitcast(F32R))
                            nc.tensor.matmul(
                                out=px[:, lc * 512:(lc + 1) * 512],
                                lhsT=wblk,
                                rhs=xt[k][:, lc * 512:(lc + 1) * 512],
                                start=(k == 0), stop=(k == 7))
                    raw = work.tile([128, HALF + 3], F32, tag="raw")
                    nc.vector.tensor_copy(out=raw[:, 0:3], in_=tails[:, ft, :])
                    nc.vector.tensor_tensor(out=raw[:, 3:], in0=px, in1=rstdB,
                                            op=AluOpType.mult)
                    nc.vector.tensor_copy(out=tails[:, ft, :],
                                          in_=raw[:, HALF:HALF + 3])
                    cv = work.tile([128, HALF], F32, tag="cv")
                    nc.vector.tensor_scalar(out=cv, in0=raw[:, 0:HALF],
                                            scalar1=convw_sb[:, ft, 0:1],
                                            scalar2=None, op0=AluOpType.mult)
                    for k in range(1, D_CONV):
                        nc.vector.scalar_tensor_tensor(
                            out=cv, in0=raw[:, k:k + HALF],
                            scalar=convw_sb[:, ft, k:k + 1], in1=cv,
                            op0=AluOpType.mult, op1=AluOpType.add)
                    dst = xcv[ft] if ft < NXT else (Bfm if ft == NXT else Cfm)
                    cvb = work.tile([128, HALF], F32, tag="cvb")
                    nc.gpsimd.tensor_scalar(out=cvb, in0=cv,
                                            scalar1=convb_sb[:, ft:ft + 1],
                                            scalar2=None, op0=AluOpType.add)
                    se = work.tile([128, HALF], F32, tag="se")
                    nc.scalar.activation(out=se, in_=cvb, func=AF.Exp,
                                         scale=-1.0)
                    nc.vector.tensor_scalar(out=se, in0=se, scalar1=1.0,
                                            scalar2=None, op0=AluOpType.add)
                    nc.vector.reciprocal(out=se, in_=se)
                    nc.gpsimd.tensor_tensor(out=dst, in0=cvb, in1=se,
                                            op=AluOpType.mult)

                # in_proj dt -> softplus -> logdA
                pdt = ps_sm1.tile([NHEADS, HALF], F32, tag="sm1")
                for lc in range(2):
                    for k in range(8):
                        wblk = wpool.tile([128, NHEADS], F32R, tag="wdt")
                        nc.sync.dma_start(
                            out=wblk,
                            in_=wxbc[k * 128:(k + 1) * 128,
                                     CONV_DIM:CONV_DIM + NHEADS].bitcast(F32R))
                        nc.tensor.matmul(out=pdt[:, lc * 512:(lc + 1) * 512],
                                         lhsT=wblk,
                                         rhs=xt[k][:, lc * 512:(lc + 1) * 512],
                                         start=(k == 0), stop=(k == 7))
                dtraw = work.tile([NHEADS, HALF], F32, tag="raw")
                nc.vector.tensor_tensor(out=dtraw, in0=pdt,
                                        in1=rstdB[:NHEADS, :], op=AluOpType.mult)
                spe = work.tile([NHEADS, HALF], F32, tag="cv")
                nc.scalar.activation(out=spe, in_=dtraw, func=AF.Exp,
                                     bias=dtb_sb)
                nc.vector.tensor_scalar(out=spe, in0=spe, scalar1=1.0,
                                        scalar2=None, op0=AluOpType.add)
                nc.scalar.activation(out=dtfm, in_=spe, func=AF.Ln)
                nc.vector.tensor_scalar(out=logda, in0=dtfm, scalar1=lamneg,
                                        scalar2=None, op0=AluOpType.mult)
                if half == 0 and DBG:
                    nc.sync.dma_start(out=DBG["dt"][:, :], in_=dtfm)

                # z projection (token-major), spilled to DRAM as bf16
                for fc2 in range(2):
                    zw = [[xin.tile([128, 512], F32R, tag=f"zw{k}_{j}", name=f"zw{half}_{fc2}_{k}_{j}")
                           for j in range(2)] for k in range(8)]
                    for k in range(8):
                        for j in range(2):
                            nc.sync.dma_start(
                                out=zw[k][j],
                                in_=wz[k * 128:(k + 1) * 128,
                                       fc2 * 1024 + j * 512:
                                       fc2 * 1024 + (j + 1) * 512].bitcast(F32R))
                    for lt in range(NCH):
                        pz = ps_in.tile([128, HALF], F32, tag="pin")
                        for j in range(2):
                            for k in range(8):
                                nc.tensor.matmul(
                                    out=pz[:, j * 512:(j + 1) * 512],
                                    lhsT=xt[k][:, lt * 128:(lt + 1) * 128],
                                    rhs=zw[k][j], start=(k == 0), stop=(k == 7))
                        zs = work.tile([128, HALF], F32, tag="cvb")
                        nc.vector.tensor_scalar(out=zs, in0=pz,
                                                scalar1=rstd_tok[:, lt:lt + 1],
                                                scalar2=None, op0=AluOpType.mult)
                        ze = work.tile([128, HALF], F32, tag="se")
                        nc.scalar.activation(out=ze, in_=zs, func=AF.Exp,
                                             scale=-1.0)
                        nc.vector.tensor_scalar(out=ze, in0=ze, scalar1=1.0,
                                                scalar2=None, op0=AluOpType.add)
                        nc.vector.reciprocal(out=ze, in_=ze)
                        z_sb = xin.tile([128, HALF], F16, tag="zsb")
                        nc.gpsimd.tensor_tensor(out=z_sb, in0=zs, in1=ze,
                                                op=AluOpType.mult)
                        nc.sync.dma_start(
                            out=z_dram[l0 + lt * 128: l0 + (lt + 1) * 128,
                                       fc2 * 1024:(fc2 + 1) * 1024],
                            in_=z_sb)

            # ================= phase 2: scan + gate + projections ==========
            with tc.tile_pool(name=f"scan{half}", bufs=1) as scan, \
                 tc.tile_pool(name=f"scs{half}", bufs=2) as scs, \
                 tc.tile_pool(name=f"swork{half}", bufs=2) as swork, \
                 tc.tile_pool(name=f"psbig{half}", bufs=1, space="PSUM") as ps_big, \
                 tc.tile_pool(name=f"psbf{half}", bufs=1, space="PSUM") as ps_bf, \
                 tc.tile_pool(name=f"pssm{half}b", bufs=2, space="PSUM") as ps_sm:
                ynbuf = [scan.tile([128, 512], F16, tag=f"ynb{t}", name=f"ynb{half}_{t}")
                         for t in range(16)]
                for ck in range(NCH):
                    gk = half * NCH + ck
                    c0 = ck * Q
                    BfmC = Bfm[:, c0:c0 + Q]
                    CfmC = Cfm[:, c0:c0 + Q]

                    # cum over chunk + decay factors
                    cumFm = scs.tile([NHEADS, Q], F32, tag="cumFm")
                    nc.vector.tensor_tensor_scan(
                        out=cumFm, data0=ones32, data1=logda[:, c0:c0 + Q],
                        initial=0.0, op0=AluOpType.mult, op1=AluOpType.add)
                    t1fm = scs.tile([NHEADS, Q], F32, tag="t1fm")
                    nc.vector.tensor_scalar(out=t1fm, in0=cumFm,
                                            scalar1=cumFm[:, Q - 1:Q],
                                            scalar2=None, op0=AluOpType.subtract)
                    sm_ps = ps_sm.tile([128, 96], F32, tag="psA")
                    nc.tensor.transpose(out=sm_ps[:, 0:32], in_=cumFm,
                                        identity=ident[:NHEADS, :NHEADS])
                    nc.tensor.transpose(out=sm_ps[:, 32:64], in_=t1fm,
                                        identity=ident[:NHEADS, :NHEADS])
                    nc.tensor.transpose(out=sm_ps[:, 64:96],
                                        in_=dtfm[:, c0:c0 + Q],
                                        identity=ident[:NHEADS, :NHEADS])
                    cumT = scs.tile([128, NHEADS], F32, tag="cumT")
                    nc.scalar.copy(out=cumT, in_=sm_ps[:, 0:32])
                    colfT = scs.tile([128, NHEADS], F16, tag="colfT")
                    nc.scalar.activation(out=colfT, in_=sm_ps[:, 32:64],
                                         func=AF.Exp, scale=-1.0)
                    etok = scs.tile([128, NHEADS], F16, tag="etok")
                    nc.scalar.activation(out=etok, in_=sm_ps[:, 0:32], func=AF.Exp)
                    dtT = scs.tile([128, NHEADS], F16, tag="dtT")
                    nc.scalar.copy(out=dtT, in_=sm_ps[:, 64:96])
                    wT = scs.tile([128, NHEADS], F16, tag="wT")
                    nc.vector.tensor_tensor(out=wT, in0=dtT, in1=colfT,
                                            op=AluOpType.mult)
                    # R = exp(chunk sum) broadcast over n partitions
                    nc.sync.dma_start(out=cend_dram[gk:gk + 1, :], in_=cumT[127:128, :])
                    cendB = scs.tile([128, NHEADS], F32, tag="cendB")
                    nc.sync.dma_start(
                        out=cendB,
                        in_=cend_dram[gk:gk + 1, :].broadcast_to([128, NHEADS]))
                    RexpB = scs.tile([128, NHEADS], F16, tag="RexpB")
                    nc.scalar.activation(out=RexpB, in_=cendB, func=AF.Exp)

                    # decay matrix for all heads: [s, (h,t)]
                    nc.sync.dma_start(
                        out=cumrow_dram[gk:gk + 1, :].rearrange(
                            "o (h t) -> (o h) t", h=NHEADS),
                        in_=cumFm)
                    cumrowB = scan.tile([128, NHEADS * Q], F32, tag="cumrowB")
                    nc.sync.dma_start(
                        out=cumrowB,
                        in_=cumrow_dram[gk:gk + 1, :].broadcast_to([128, NHEADS * Q]))
                    dall = scan.tile([128, NHEADS * Q], F32, tag="dall")
                    nc.vector.tensor_tensor(
                        out=dall.rearrange("p (h t) -> p h t", h=NHEADS),
                        in0=cumrowB.rearrange("p (h t) -> p h t", h=NHEADS),
                        in1=cumT[:, :, None].broadcast_to([128, NHEADS, Q]),
                        op=AluOpType.subtract)
                    nc.gpsimd.tensor_scalar(out=dall, in0=dall, scalar1=0.0,
                                            scalar2=None, op0=AluOpType.min)
                    e_bf = scan.tile([128, NHEADS * Q], F16, tag="cumrowB")
                    nc.scalar.activation(out=e_bf, in_=dall, func=AF.Exp)

                    if gk == 0 and DBG:
                        nc.sync.dma_start(out=DBG["cum"][:, :], in_=cumFm)
                        dbgt = swork.tile([128, 512], F32, tag="dbgt")
                        nc.vector.tensor_copy(out=dbgt[:, 0:Q], in_=xcv[0][:, 0:Q])
                        nc.sync.dma_start(out=DBG["x0"][:, :], in_=dbgt[:, 0:Q])
                        nc.vector.tensor_copy(out=dbgt[:, 0:Q], in_=Bfm[:, 0:Q])
                        nc.sync.dma_start(out=DBG["B"][:, :], in_=dbgt[:, 0:Q])
                    gp = ps_sm.tile([Q, Q], F32, tag="psA")
                    nc.tensor.matmul(out=gp, lhsT=BfmC, rhs=CfmC,
                                     start=True, stop=True)
                    gm = scs.tile([Q, Q], F16, tag="gm")
                    nc.vector.tensor_tensor(out=gm, in0=gp, in1=tri_bf,
                                            op=AluOpType.mult)
                    m_bf = scan.tile([128, NHEADS * Q], F16, tag="dall")
                    nc.vector.tensor_tensor(
                        out=m_bf.rearrange("p (h t) -> p h t", h=NHEADS),
                        in0=e_bf.rearrange("p (h t) -> p h t", h=NHEADS),
                        in1=gm[:, None, :].broadcast_to([Q, NHEADS, Q]),
                        op=AluOpType.mult)

                    if gk == 0 and DBG:
                        dbgt = swork.tile([128, 512], F32, tag="dbgt")
                        nc.vector.tensor_copy(out=dbgt, in_=e_bf[:, 0:512])
                        nc.sync.dma_start(out=DBG["e"][:, :], in_=dbgt)
                        nc.vector.tensor_copy(out=dbgt, in_=m_bf[:, 0:512])
                        nc.sync.dma_start(out=DBG["m"][:, :], in_=dbgt)
                    # token-major x for this chunk
                    xt_ps = ps_bf.tile([128, D_INNER], F16, tag="xtps")
                    for i in range(NXT):
                        nc.tensor.transpose(out=xt_ps[:, i * 128:(i + 1) * 128],
                                            in_=xcv[i][:, c0:c0 + Q],
                                            identity=ident_bf)
                    xtok = scan.tile([128, D_INNER], F16, tag="xtok")
                    for j in range(4):
                        nc.scalar.copy(out=xtok[:, j * 512:(j + 1) * 512],
                                       in_=xt_ps[:, j * 512:(j + 1) * 512])
                    if gk == 0 and DBG:
                        dbgt = swork.tile([128, 512], F32, tag="dbgt")
                        nc.vector.tensor_copy(out=dbgt, in_=xtok[:, 0:512])
                        nc.sync.dma_start(out=DBG["xtok"][:, :], in_=dbgt)
                    xd = scan.tile([128, D_INNER], F16, tag="xd")
                    nc.vector.tensor_tensor(
                        out=xd.rearrange("p (h q) -> p h q", h=NHEADS),
                        in0=xtok.rearrange("p (h q) -> p h q", h=NHEADS),
                        in1=dtT[:, :, None].broadcast_to([128, NHEADS, HEADDIM]),
                        op=AluOpType.mult)
                    xw = scan.tile([128, D_INNER], F16, tag="xw")
                    nc.gpsimd.tensor_tensor(
                        out=xw.rearrange("p (h q) -> p h q", h=NHEADS),
                        in0=xtok.rearrange("p (h q) -> p h q", h=NHEADS),
                        in1=wT[:, :, None].broadcast_to([128, NHEADS, HEADDIM]),
                        op=AluOpType.mult)
                    s_scaled = scan.tile([D_STATE, D_INNER], F16, tag="s_scaled")
                    nc.gpsimd.tensor_tensor(
                        out=s_scaled.rearrange("p (h q) -> p h q", h=NHEADS),
                        in0=S_bf[sidx].rearrange("p (h q) -> p h q", h=NHEADS),
                        in1=RexpB[:, :, None].broadcast_to([128, NHEADS, HEADDIM]),
                        op=AluOpType.mult)

                    # inter-chunk contribution + D-skip
                    yi_ps = ps_big.tile([Q, D_INNER], F32, tag="ybig")
                    for j in range(4):
                        nc.tensor.matmul(out=yi_ps[:, j * 512:(j + 1) * 512],
                                         lhsT=CfmC,
                                         rhs=S_bf[sidx][:, j * 512:(j + 1) * 512],
                                         start=True, stop=True)
                    yacc = scan.tile([Q, D_INNER], F32, tag="yacc")
                    nc.vector.tensor_tensor(
                        out=yacc.rearrange("p (h q) -> p h q", h=NHEADS),
                        in0=yi_ps.rearrange("p (h q) -> p h q", h=NHEADS),
                        in1=etok[:, :, None].broadcast_to([128, NHEADS, HEADDIM]),
                        op=AluOpType.mult)
                    dskip = scan.tile([Q, D_INNER], F16, tag="dskip")
                    nc.gpsimd.tensor_tensor(
                        out=dskip.rearrange("p (h q) -> p h q", h=NHEADS),
                        in0=xtok.rearrange("p (h q) -> p h q", h=NHEADS),
                        in1=d_bc[:, :, None].broadcast_to([128, NHEADS, HEADDIM]),
                        op=AluOpType.mult)
                    acc = scan.tile([Q, D_INNER], F32, tag="t3")
                    nc.gpsimd.tensor_tensor(out=acc, in0=yacc, in1=dskip,
                                            op=AluOpType.add)

                    # intra-chunk attention (per head)
                    ya_ps = ps_big.tile([Q, D_INNER], F32, tag="ybig")
                    for h in range(NHEADS):
                        nc.tensor.matmul(
                            out=ya_ps[:, h * HEADDIM:(h + 1) * HEADDIM],
                            lhsT=m_bf[:, h * Q:(h + 1) * Q],
                            rhs=xd[:, h * HEADDIM:(h + 1) * HEADDIM],
                            start=True, stop=True)
                    t3 = scan.tile([Q, D_INNER], F32, tag="yacc")
                    nc.vector.tensor_tensor(out=t3, in0=ya_ps, in1=acc,
                                            op=AluOpType.add)

                    # state update
                    bt_ps = ps_sm.tile([Q, D_STATE], F16, tag="psA")
                    nc.tensor.transpose(out=bt_ps, in_=BfmC, identity=ident_bf)
                    btok = scs.tile([Q, D_STATE], F16, tag="btok")
                    nc.scalar.copy(out=btok, in_=bt_ps)
                    s_ps = ps_big.tile([D_STATE, D_INNER], F32, tag="ybig")
                    for j in range(4):
                        nc.tensor.matmul(out=s_ps[:, j * 512:(j + 1) * 512],
                                         lhsT=btok,
                                         rhs=xw[:, j * 512:(j + 1) * 512],
                                         start=True, stop=True)
                    nc.vector.tensor_tensor(out=S_bf[1 - sidx], in0=s_ps,
                                            in1=s_scaled, op=AluOpType.add)
                    sidx = 1 - sidx

                    # gate + rmsnorm
                    zchunk = swork.tile([Q, D_INNER], F16, tag="zchunk")
                    nc.sync.dma_start(out=zchunk,
                                      in_=z_dram[l0 + c0: l0 + c0 + Q, :])
                    if gk == 0 and DBG:
                        nc.sync.dma_start(out=DBG["t3"][:, :], in_=t3[:, 0:512])
                    yg = scan.tile([Q, D_INNER], F32, tag="t3")
                    nc.vector.tensor_tensor(out=yg, in0=t3, in1=zchunk,
                                            op=AluOpType.mult)
                    sq2 = swork.tile([Q, D_INNER], F16, tag="sq2")
                    ssq2 = scs.tile([Q, 1], F32, tag="ssq2")
                    nc.scalar.activation(out=sq2, in_=yg, func=AF.Square,
                                         accum_out=ssq2)
                    lny = scs.tile([Q, 1], F32, tag="lny")
                    nc.scalar.activation(out=lny, in_=ssq2, func=AF.Ln,
                                         bias=eps2, scale=1.0 / D_INNER)
                    rstd2 = scs.tile([Q, 1], F32, tag="rstd2")
                    nc.scalar.activation(out=rstd2, in_=lny, func=AF.Exp,
                                         scale=-0.5)
                    yn = scan.tile([Q, D_INNER], F16, tag="yn")
                    nc.vector.tensor_scalar(out=yn, in0=yg, scalar1=rstd2,
                                            scalar2=None, op0=AluOpType.mult)

                    if gk == 0 and DBG:
                        dbgt = swork.tile([128, 512], F32, tag="dbgt")
                        nc.vector.tensor_copy(out=dbgt, in_=yn[:, 0:512])
                        nc.sync.dma_start(out=DBG["yn"][:, :], in_=dbgt)
                    # transpose y_n to feature-major into the l-group buffer
                    yt_ps = ps_bf.tile([128, D_INNER], F16, tag="xtps")
                    for i in range(16):
                        nc.tensor.transpose(out=yt_ps[:, i * 128:(i + 1) * 128],
                                            in_=yn[:, i * 128:(i + 1) * 128],
                                            identity=ident_bf)
                    gcol = (gk % 4) * 128
                    for i in range(16):
                        nc.scalar.copy(out=ynbuf[i][:, gcol:gcol + 128],
                                       in_=yt_ps[:, i * 128:(i + 1) * 128])

                    # every 4 chunks: out_proj + final projection + store
                    if gk % 4 == 3:
                        g = gk // 4
                        out1 = [scan.tile([128, 512], F16, tag=f"out1{m}", name=f"out1_{gk}_{m}")
                                for m in range(8)]
                        for m in range(8):
                            op_ps = ps_sm.tile([128, 512], F32, tag="psA")
                            for kt in range(16):
                                wo = wpool.tile([128, 128], F16, tag="wo")
                                nc.sync.dma_start(
                                    out=wo,
                                    in_=wout[kt * 128:(kt + 1) * 128,
                                             m * 128:(m + 1) * 128])
                                nc.tensor.matmul(out=op_ps, lhsT=wo,
                                                 rhs=ynbuf[kt],
                                                 start=(kt == 0), stop=(kt == 15))
                            nc.scalar.copy(out=out1[m], in_=op_ps)
                        for lt2 in range(4):
                            for j in range(2):
                                o_ps = ps_sm.tile([128, 512], F32, tag="psA")
                                for m in range(8):
                                    nc.tensor.matmul(
                                        out=o_ps,
                                        lhsT=out1[m][:, lt2 * 128:(lt2 + 1) * 128],
                                        rhs=wp_sb[m][:, j * 512:(j + 1) * 512],
                                        start=(m == 0), stop=(m == 7))
                                o_sb = swork.tile([128, 512], F32, tag="osb")
                                nc.scalar.copy(out=o_sb, in_=o_ps)
                                nc.sync.dma_start(
                                    out=o[g * 512 + lt2 * 128:
                                          g * 512 + (lt2 + 1) * 128,
                                          j * 512:(j + 1) * 512],
                                    in_=o_sb)


# ---------------------------------------------------------------------------
# Host wrapper
# ---------------------------------------------------------------------------
_NC_CACHE = None


def _get_program():
    global _NC_CACHE
    if _NC_CACHE is None:
        _NC_CACHE = build_program()
    return _NC_CACHE


def _per_core_inputs(inputs):
    x = np.asarray(inputs["x"], np.float32)
    norm_w = np.asarray(inputs["norm_w"], np.float32)
    tri = np.triu(np.ones((Q, Q), np.float32)).astype(np.float16)
    ident = np.eye(128, dtype=np.float32)
    in_maps = []
    shared = {}
    for dirn, pre in enumerate(("fwd_", "bwd_")):
        in_W = np.asarray(inputs[pre + "in_W"], np.float32) * norm_w[None, :]
        wxbc = np.ascontiguousarray(in_W[D_INNER:].T)
        wz = np.ascontiguousarray(in_W[:D_INNER].T)
        out_W = (np.asarray(inputs[pre + "out_W"], np.float32)
                 * np.asarray(inputs[pre + "gnorm_w"], np.float32)[None, :])
        wout = np.ascontiguousarray(out_W.T).astype(np.float16)
        pW = np.asarray(inputs["proj_W"], np.float32)
        wp_half = np.ascontiguousarray(
            pW[:, dirn * D_MODEL:(dirn + 1) * D_MODEL].T).astype(np.float16)
        shared[dirn] = dict(
            wxbc=wxbc, wz=wz, wout=wout, wp=wp_half,
            convw=np.asarray(inputs[pre + "conv_w"], np.float32),
            convb=np.asarray(inputs[pre + "conv_b"], np.float32),
            dtb=np.asarray(inputs[pre + "dt_bias"], np.float32),
            alog=np.asarray(inputs[pre + "A_log"], np.float32),
            dvec=np.asarray(inputs[pre + "D"], np.float32),
            tri=tri, identw=ident,
        )
    for c in range(8):
        b, dirn = c % 4, c // 4
        xb = x[b] if dirn == 0 else x[b, ::-1]
        m = dict(shared[dirn])
        m["xT"] = np.ascontiguousarray(xb.T)
        in_maps.append(m)
    return in_maps


def run_cores(inputs, trace=False):
    nc = _get_program()
    in_maps = _per_core_inputs(inputs)
    return run_bass_kernel_spmd(nc, in_maps, list(range(8)), trace=trace)


def kernel(**inputs):
    res = run_cores(inputs)
    x = np.asarray(inputs["x"], np.float32)
    proj_b = np.asarray(inputs["proj_b"], np.float32)
    out = np.empty_like(x)
    for b in range(4):
        out[b] = (x[b] + proj_b
                  + res.results[b]["o"]
                  + res.results[4 + b]["o"][::-1])
    return out


# revision 13
# speedup vs baseline: 1.6691x; 1.6691x over previous
"""BiMamba block kernel for Trainium2, 8 NeuronCores.

Sharding: 8 cores = 4 batches x 2 directions (fwd/bwd). Each core runs the
full mamba2 for one (batch, direction) with the sequence pre-flipped on the
host for bwd cores, and computes its half of the final projection. The host
combines: out[b] = x[b] + proj_b + part_fwd[b] + flip(part_bwd[b]).

Per-core pipeline (l=2048 processed in 2 halves of 1024):
  in_proj (f32r matmuls, x-rmsnorm folded in at psum evacuation) -> causal
  depthwise conv + silu (feature-major) -> chunked SSD selective scan
  (Q=128 chunks; matmul-based intra-chunk attention with difference-form
  decay, heads batched where operands are shared) -> gate + rmsnorm ->
  out_proj -> final proj half (bf16 matmuls).
"""
import contextlib

import numpy as np
import ml_dtypes

import bass_rust
import concourse.bass as bass
import concourse.mybir as mybir
import concourse.tile as tile
from concourse.alu_op_type import AluOpType
from concourse.bass_utils import run_bass_kernel_spmd
from concourse.vector_clock import ScopedClock

F32 = mybir.dt.float32
F32R = mybir.dt.float32r
BF16 = mybir.dt.bfloat16
F16 = mybir.dt.float16
AF = mybir.ActivationFunctionType

D_MODEL = 1024
D_STATE = 128
D_CONV = 4
HEADDIM = 64
D_INNER = 2048
NHEADS = 32
CONV_DIM = D_INNER + 2 * D_STATE          # 2304
L = 2048
Q = 128                                    # scan chunk
HALF = 1024                                # seq processed per pass
NCH = HALF // Q                            # chunks per half
NXT = D_INNER // 128                       # 16 x-part feature tiles
NBCT = CONV_DIM // 128                     # 18 conv feature tiles


# ---------------------------------------------------------------------------
# Tile/walrus compatibility patches: this toolchain's walrus rejects >1 sync
# wait per instruction; split extra waits onto same-engine nops.
# ---------------------------------------------------------------------------
def _split_drain_and_barrier(self, tick_clock, wait_clock):
    nc = self.nc
    drain_inst = nc.sync.drain()
    wait_clock.add_sem_waits(
        drain_inst.ins, ScopedClock({None: tick_clock.global_clock})
    )
    si = drain_inst.ins.sync_info
    waits = list(si.on_wait or []) if si is not None else []
    if len(waits) > 1:
        si.on_wait = waits[:1]
        drain_inst.ins.sync_info = si
        for i in range(1, len(waits)):
            n2 = nc.sync.nop()
            n2.ins.sync_info = bass_rust.SyncInfo(on_wait=[waits[i]], on_update=[])
    nc.all_engine_barrier()
    assert self.sems is not None
    popped = nc._tile_sem_poison_stack.pop()
    assert popped is self._sem_poison
    nc.clear_and_free_semaphores(list(self.sems.allocated().values()))
    nc.all_engine_barrier()


tile.TileContext._drain_and_barrier = _split_drain_and_barrier


def split_multi_waits(nc, maxw=1):
    cnt = 0
    for f in nc.m.functions:
        for bb in f.blocks:
            insts = bb.instructions
            i = 0
            while i < len(insts):
                inst = insts[i]
                si = inst.sync_info
                waits = list(si.on_wait) if (si is not None and si.on_wait) else []
                if len(waits) > maxw:
                    si.on_wait = waits[:maxw]
                    inst.sync_info = si
                    for j in range(maxw, len(waits), maxw):
                        n = mybir.InstNoOp(name=f"I-wsplit-{cnt}")
                        cnt += 1
                        n.engine = inst.engine
                        n.sync_info = bass_rust.SyncInfo(
                            on_wait=waits[j : j + maxw], on_update=[]
                        )
                        insts.insert(i, n)
                        i += 1
                i += 1
    return cnt


# ---------------------------------------------------------------------------
# Device program (identical on all 8 cores; data differs per core)
# ---------------------------------------------------------------------------
def build_program():
    nc = bass.Bass(target_bir_lowering=False)

    xT = nc.dram_tensor("xT", [D_MODEL, L], F32, kind="ExternalInput")
    wxbc = nc.dram_tensor("wxbc", [D_MODEL, CONV_DIM + NHEADS], F32,
                          kind="ExternalInput")
    wz = nc.dram_tensor("wz", [D_MODEL, D_INNER], F32, kind="ExternalInput")
    wout = nc.dram_tensor("wout", [D_INNER, D_MODEL], F16, kind="ExternalInput")
    wp = nc.dram_tensor("wp", [D_MODEL, D_MODEL], F16, kind="ExternalInput")
    convw = nc.dram_tensor("convw", [CONV_DIM, D_CONV], F32, kind="ExternalInput")
    convb = nc.dram_tensor("convb", [CONV_DIM], F32, kind="ExternalInput")
    dtb = nc.dram_tensor("dtb", [NHEADS], F32, kind="ExternalInput")
    alog = nc.dram_tensor("alog", [NHEADS], F32, kind="ExternalInput")
    dvec = nc.dram_tensor("dvec", [NHEADS], F32, kind="ExternalInput")
    tri = nc.dram_tensor("tri", [Q, Q], F16, kind="ExternalInput")
    identw = nc.dram_tensor("identw", [128, 128], F32, kind="ExternalInput")
    o = nc.dram_tensor("o", [L, D_MODEL], F32, kind="ExternalOutput")

    z_dram = nc.dram_tensor("z_dram", [L, D_INNER], F16)
    rstd_dram = nc.dram_tensor("rstd_dram", [2, HALF], F32)
    cumrow_dram = nc.dram_tensor("cumrow_dram", [16, NHEADS * Q], F32)
    cend_dram = nc.dram_tensor("cend_dram", [16, NHEADS], F32)

    with tile.TileContext(nc) as tc:
        _build_body(nc, tc, xT, wxbc, wz, wout, wp, convw, convb, dtb, alog,
                    dvec, tri, identw, o, z_dram, rstd_dram, cumrow_dram,
                    cend_dram)
    split_multi_waits(nc)
    return nc


def _build_body(nc, tc, xT, wxbc, wz, wout, wp, convw, convb, dtb, alog,
                dvec, tri, identw, o, z_dram, rstd_dram, cumrow_dram,
                cend_dram, DBG=None):
    with contextlib.ExitStack() as ctx:
        consts = ctx.enter_context(tc.tile_pool(name="consts", bufs=1))
        persist = ctx.enter_context(tc.tile_pool(name="persist", bufs=1))
        wpool = ctx.enter_context(tc.tile_pool(name="wpool", bufs=4))

        # ---- constants
        ident = consts.tile([128, 128], F32)
        nc.sync.dma_start(out=ident, in_=identw[:, :])
        ident_bf = consts.tile([128, 128], F16)
        nc.vector.tensor_copy(out=ident_bf, in_=ident)
        tri_bf = consts.tile([Q, Q], F16)
        nc.sync.dma_start(out=tri_bf, in_=tri[:, :])
        ones_f16 = consts.tile([128, 1], F16)
        nc.vector.memset(ones_f16, 1.0)
        ones32 = consts.tile([NHEADS, Q], F32)
        nc.vector.memset(ones32, 1.0)
        eps1 = consts.tile([1, 1], F32)
        nc.vector.memset(eps1, 1e-4)
        eps2 = consts.tile([128, 1], F32)
        nc.vector.memset(eps2, 1e-5)

        dtb_sb = consts.tile([NHEADS, 1], F32)
        nc.sync.dma_start(out=dtb_sb, in_=dtb[:, None])
        alog_sb = consts.tile([NHEADS, 1], F32)
        nc.sync.dma_start(out=alog_sb, in_=alog[:, None])
        lam = consts.tile([NHEADS, 1], F32)
        nc.scalar.activation(out=lam, in_=alog_sb, func=AF.Exp)
        lamneg = consts.tile([NHEADS, 1], F32)
        nc.vector.tensor_scalar(out=lamneg, in0=lam, scalar1=-1.0, scalar2=None,
                                op0=AluOpType.mult)
        d_bc = consts.tile([128, NHEADS], F32)
        nc.sync.dma_start(out=d_bc, in_=dvec[None, :].broadcast_to([128, NHEADS]))

        convb_sb = consts.tile([128, NBCT], F32)
        nc.sync.dma_start(out=convb_sb, in_=convb.rearrange("(t p) -> p t", p=128))
        convw_sb = consts.tile([128, NBCT, D_CONV], F32)
        nc.sync.dma_start(out=convw_sb,
                          in_=convw.rearrange("(t p) k -> p t k", p=128))

        # final-proj weights resident (bf16, 2 MB)
        wp_sb = [persist.tile([128, D_MODEL], F16, tag=f"wp{k}", name=f"wp{k}") for k in range(8)]
        for k in range(8):
            nc.sync.dma_start(out=wp_sb[k], in_=wp[k * 128:(k + 1) * 128, :])

        # cross-half state
        S_bf = [persist.tile([D_STATE, D_INNER], F16, tag=f"S{i}", name=f"S{i}") for i in range(2)]
        nc.vector.memset(S_bf[0], 0.0)
        tails = persist.tile([128, NBCT, D_CONV - 1], F32)
        nc.vector.memset(tails, 0.0)

        # per-half feature-major activations (overwritten each half)
        xcv = [persist.tile([128, HALF], F16, tag=f"xcv{t}", name=f"xcv{t}") for t in range(NXT)]
        Bfm = persist.tile([128, HALF], F16, tag="Bfm")
        Cfm = persist.tile([128, HALF], F16, tag="Cfm")
        dtfm = persist.tile([NHEADS, HALF], F32, tag="dtfm")
        logda = persist.tile([NHEADS, HALF], F32, tag="logda")

        sidx = 0  # S ping-pong index

        for half in range(2):
            l0 = half * HALF

            # ================= phase 1: in_proj + conv + z =================
            with tc.tile_pool(name=f"xin{half}", bufs=1) as xin, \
                 tc.tile_pool(name=f"work{half}", bufs=2) as work, \
                 tc.tile_pool(name=f"psin{half}", bufs=3, space="PSUM") as ps_in, \
                 tc.tile_pool(name=f"pssm{half}", bufs=1, space="PSUM") as ps_sm1:
                xt = [xin.tile([128, HALF], F32R, tag=f"xt{k}", name=f"xt{half}_{k}") for k in range(8)]
                for k in range(8):
                    nc.sync.dma_start(
                        out=xt[k],
                        in_=xT[k * 128:(k + 1) * 128, l0:l0 + HALF].bitcast(F32R))

                # rmsnorm(x): column sums of squares via PE ones-reduce
                ssq_ps = ps_sm1.tile([1, HALF], F32, tag="sm1")
                for k in range(8):
                    sq = work.tile([128, HALF], F16, tag="sq")
                    nc.scalar.activation(out=sq, in_=xt[k].bitcast(F32),
                                         func=AF.Square)
                    for lc in range(2):
                        nc.tensor.matmul(out=ssq_ps[:, lc * 512:(lc + 1) * 512],
                                         lhsT=ones_f16,
                                         rhs=sq[:, lc * 512:(lc + 1) * 512],
                                         start=(k == 0), stop=(k == 7))
                lnv = work.tile([1, HALF], F32, tag="lnv")
                nc.scalar.activation(out=lnv, in_=ssq_ps, func=AF.Ln,
                                     bias=eps1, scale=1.0 / D_MODEL)
                rstd_row = work.tile([1, HALF], F32, tag="rstdrow")
                nc.scalar.activation(out=rstd_row, in_=lnv, func=AF.Exp,
                                     scale=-0.5)
                nc.sync.dma_start(out=rstd_dram[half:half + 1, :], in_=rstd_row)
                rstdB = xin.tile([128, HALF], F32, tag="rstdB")
                nc.sync.dma_start(out=rstdB,
                                  in_=rstd_dram[half:half + 1, :].broadcast_to([128, HALF]))
                rstd_tok = work.tile([128, NCH], F32, tag="rstdtok")
                nc.sync.dma_start(
                    out=rstd_tok,
                    in_=rstd_dram[half:half + 1, :].rearrange("o (t p) -> (o p) t", p=128))

                # in_proj xBC (feature-major) + conv + silu
                for ft in range(NBCT):
                    px = ps_in.tile([128, HALF], F32, tag="pin")
                    for lc in range(2):
                        for k in range(8):
                            wblk = wpool.tile([128, 128], F32R, tag="wblk")
                            nc.sync.dma_start(
                                out=wblk,
                                in_=wxbc[k * 128:(k + 1) * 128,
                                         ft * 128:(ft + 1) * 128].bitcast(F32R))
                            nc.tensor.matmul(
                                out=px[:, lc * 512:(lc + 1) * 512],
                                lhsT=wblk,
                                rhs=xt[k][:, lc * 512:(lc + 1) * 512],
                                start=(k == 0), stop=(k == 7))
                    raw = work.tile([128, HALF + 3], F32, tag="raw")
                    nc.vector.tensor_copy(out=raw[:, 0:3], in_=tails[:, ft, :])
                    nc.vector.tensor_tensor(out=raw[:, 3:], in0=px, in1=rstdB,
                                            op=AluOpType.mult)
                    nc.vector.tensor_copy(out=tails[:, ft, :],
                                          in_=raw[:, HALF:HALF + 3])
                    cv = work.tile([128, HALF], F32, tag="cv")
                    nc.vector.tensor_scalar(out=cv, in0=raw[:, 0:HALF],
                                            scalar1=convw_sb[:, ft, 0:1],
                                            scalar2=None, op0=AluOpType.mult)
                    for k in range(1, D_CONV):
                        nc.vector.scalar_tensor_tensor(
                            out=cv, in0=raw[:, k:k + HALF],
                            scalar=convw_sb[:, ft, k:k + 1], in1=cv,
                            op0=AluOpType.mult, op1=AluOpType.add)
                    dst = xcv[ft] if ft < NXT else (Bfm if ft == NXT else Cfm)
                    cvb = work.tile([128, HALF], F32, tag="cvb")
                    nc.gpsimd.tensor_scalar(out=cvb, in0=cv,
                                            scalar1=convb_sb[:, ft:ft + 1],
                                            scalar2=None, op0=AluOpType.add)
                    se = work.tile([128, HALF], F32, tag="se")
                    nc.scalar.activation(out=se, in_=cvb, func=AF.Exp,
                                         scale=-1.0)
                    nc.vector.tensor_scalar(out=se, in0=se, scalar1=1.0,
                                            scalar2=None, op0=AluOpType.add)
                    nc.vector.reciprocal(out=se, in_=se)
                    nc.gpsimd.tensor_tensor(out=dst, in0=cvb, in1=se,
                                            op=AluOpType.mult)

                # in_proj dt -> softplus -> logdA
                pdt = ps_sm1.tile([NHEADS, HALF], F32, tag="sm1")
                for lc in range(2):
                    for k in range(8):
                        wblk = wpool.tile([128, NHEADS], F32R, tag="wdt")
                        nc.sync.dma_start(
                            out=wblk,
                            in_=wxbc[k * 128:(k + 1) * 128,
                                     CONV_DIM:CONV_DIM + NHEADS].bitcast(F32R))
                        nc.tensor.matmul(out=pdt[:, lc * 512:(lc + 1) * 512],
                                         lhsT=wblk,
                                         rhs=xt[k][:, lc * 512:(lc + 1) * 512],
                                         start=(k == 0), stop=(k == 7))
                dtraw = work.tile([NHEADS, HALF], F32, tag="raw")
                nc.vector.tensor_tensor(out=dtraw, in0=pdt,
                                        in1=rstdB[:NHEADS, :], op=AluOpType.mult)
                spe = work.tile([NHEADS, HALF], F32, tag="cv")
                nc.scalar.activation(out=spe, in_=dtraw, func=AF.Exp,
                                     bias=dtb_sb)
                nc.vector.tensor_scalar(out=spe, in0=spe, scalar1=1.0,
                                        scalar2=None, op0=AluOpType.add)
                nc.scalar.activation(out=dtfm, in_=spe, func=AF.Ln)
                nc.vector.tensor_scalar(out=logda, in0=dtfm, scalar1=lamneg,
                                        scalar2=None, op0=AluOpType.mult)

                # z projection (token-major), spilled to DRAM as bf16
                for fc2 in range(2):
                    zw = [[xin.tile([128, 512], F32R, tag=f"zw{k}_{j}", name=f"zw{half}_{fc2}_{k}_{j}")
                           for j in range(2)] for k in range(8)]
                    for k in range(8):
                        for j in range(2):
                            nc.sync.dma_start(
                                out=zw[k][j],
                                in_=wz[k * 128:(k + 1) * 128,
                                       fc2 * 1024 + j * 512:
                                       fc2 * 1024 + (j + 1) * 512].bitcast(F32R))
                    for lt in range(NCH):
                        pz = ps_in.tile([128, HALF], F32, tag="pin")
                        for j in range(2):
                            for k in range(8):
                                nc.tensor.matmul(
                                    out=pz[:, j * 512:(j + 1) * 512],
                                    lhsT=xt[k][:, lt * 128:(lt + 1) * 128],
                                    rhs=zw[k][j], start=(k == 0), stop=(k == 7))
                        zs = work.tile([128, HALF], F32, tag="cvb")
                        nc.vector.tensor_scalar(out=zs, in0=pz,
                                                scalar1=rstd_tok[:, lt:lt + 1],
                                                scalar2=None, op0=AluOpType.mult)
                        ze = work.tile([128, HALF], F32, tag="se")
                        nc.scalar.activation(out=ze, in_=zs, func=AF.Exp,
                                             scale=-1.0)
                        nc.vector.tensor_scalar(out=ze, in0=ze, scalar1=1.0,
                                                scalar2=None, op0=AluOpType.add)
                        nc.vector.reciprocal(out=ze, in_=ze)
                        z_sb = xin.tile([128, HALF], F16, tag="zsb")
                        nc.gpsimd.tensor_tensor(out=z_sb, in0=zs, in1=ze,
                                                op=AluOpType.mult)
                        nc.sync.dma_start(
                            out=z_dram[l0 + lt * 128: l0 + (lt + 1) * 128,
                                       fc2 * 1024:(fc2 + 1) * 1024],
                            in_=z_sb)

            # ================= phase 2: scan + gate + projections ==========
            with tc.tile_pool(name=f"scan{half}", bufs=1) as scan, \
                 tc.tile_pool(name=f"scs{half}", bufs=2) as scs, \
                 tc.tile_pool(name=f"swork{half}", bufs=2) as swork, \
                 tc.tile_pool(name=f"psbig{half}", bufs=1, space="PSUM") as ps_big, \
                 tc.tile_pool(name=f"psbf{half}", bufs=1, space="PSUM") as ps_bf, \
                 tc.tile_pool(name=f"pssm{half}b", bufs=2, space="PSUM") as ps_sm:
                ynbuf = [scan.tile([128, 512], F16, tag=f"ynb{t}", name=f"ynb{half}_{t}")
                         for t in range(16)]
                for ck in range(NCH):
                    gk = half * NCH + ck
                    c0 = ck * Q
                    BfmC = Bfm[:, c0:c0 + Q]
                    CfmC = Cfm[:, c0:c0 + Q]

                    # cum over chunk + decay factors
                    cumFm = scs.tile([NHEADS, Q], F32, tag="cumFm")
                    nc.vector.tensor_tensor_scan(
                        out=cumFm, data0=ones32, data1=logda[:, c0:c0 + Q],
                        initial=0.0, op0=AluOpType.mult, op1=AluOpType.add)
                    t1fm = scs.tile([NHEADS, Q], F32, tag="t1fm")
                    nc.vector.tensor_scalar(out=t1fm, in0=cumFm,
                                            scalar1=cumFm[:, Q - 1:Q],
                                            scalar2=None, op0=AluOpType.subtract)
                    sm_ps = ps_sm.tile([128, 96], F32, tag="psA")
                    nc.tensor.transpose(out=sm_ps[:, 0:32], in_=cumFm,
                                        identity=ident[:NHEADS, :NHEADS])
                    nc.tensor.transpose(out=sm_ps[:, 32:64], in_=t1fm,
                                        identity=ident[:NHEADS, :NHEADS])
                    nc.tensor.transpose(out=sm_ps[:, 64:96],
                                        in_=dtfm[:, c0:c0 + Q],
                                        identity=ident[:NHEADS, :NHEADS])
                    cumT = scs.tile([128, NHEADS], F32, tag="cumT")
                    nc.scalar.copy(out=cumT, in_=sm_ps[:, 0:32])
                    colfT = scs.tile([128, NHEADS], F16, tag="colfT")
                    nc.scalar.activation(out=colfT, in_=sm_ps[:, 32:64],
                                         func=AF.Exp, scale=-1.0)
                    etok = scs.tile([128, NHEADS], F16, tag="etok")
                    nc.scalar.activation(out=etok, in_=sm_ps[:, 0:32], func=AF.Exp)
                    dtT = scs.tile([128, NHEADS], F16, tag="dtT")
                    nc.scalar.copy(out=dtT, in_=sm_ps[:, 64:96])
                    wT = scs.tile([128, NHEADS], F16, tag="wT")
                    nc.vector.tensor_tensor(out=wT, in0=dtT, in1=colfT,
                                            op=AluOpType.mult)
                    # R = exp(chunk sum) broadcast over n partitions
                    nc.sync.dma_start(out=cend_dram[gk:gk + 1, :], in_=cumT[127:128, :])
                    cendB = scs.tile([128, NHEADS], F32, tag="cendB")
                    nc.sync.dma_start(
                        out=cendB,
                        in_=cend_dram[gk:gk + 1, :].broadcast_to([128, NHEADS]))
                    RexpB = scs.tile([128, NHEADS], F16, tag="RexpB")
                    nc.scalar.activation(out=RexpB, in_=cendB, func=AF.Exp)

                    # decay matrix for all heads: [s, (h,t)]
                    nc.sync.dma_start(
                        out=cumrow_dram[gk:gk + 1, :].rearrange(
                            "o (h t) -> (o h) t", h=NHEADS),
                        in_=cumFm)
                    cumrowB = scan.tile([128, NHEADS * Q], F32, tag="cumrowB")
                    nc.sync.dma_start(
                        out=cumrowB,
                        in_=cumrow_dram[gk:gk + 1, :].broadcast_to([128, NHEADS * Q]))
                    dall = scan.tile([128, NHEADS * Q], F32, tag="dall")
                    nc.vector.tensor_tensor(
                        out=dall.rearrange("p (h t) -> p h t", h=NHEADS),
                        in0=cumrowB.rearrange("p (h t) -> p h t", h=NHEADS),
                        in1=cumT[:, :, None].broadcast_to([128, NHEADS, Q]),
                        op=AluOpType.subtract)
                    nc.gpsimd.tensor_scalar(out=dall, in0=dall, scalar1=0.0,
                                            scalar2=None, op0=AluOpType.min)
                    e_bf = scan.tile([128, NHEADS * Q], F16, tag="cumrowB")
                    nc.scalar.activation(out=e_bf, in_=dall, func=AF.Exp)

                    gp = ps_sm.tile([Q, Q], F32, tag="psA")
                    nc.tensor.matmul(out=gp, lhsT=BfmC, rhs=CfmC,
                                     start=True, stop=True)
                    gm = scs.tile([Q, Q], F16, tag="gm")
                    nc.vector.tensor_tensor(out=gm, in0=gp, in1=tri_bf,
                                            op=AluOpType.mult)
                    m_bf = scan.tile([128, NHEADS * Q], F16, tag="dall")
                    nc.vector.tensor_tensor(
                        out=m_bf.rearrange("p (h t) -> p h t", h=NHEADS),
                        in0=e_bf.rearrange("p (h t) -> p h t", h=NHEADS),
                        in1=gm[:, None, :].broadcast_to([Q, NHEADS, Q]),
                        op=AluOpType.mult)

                    # token-major x for this chunk
                    xt_ps = ps_bf.tile([128, D_INNER], F16, tag="xtps")
                    for i in range(NXT):
                        nc.tensor.transpose(out=xt_ps[:, i * 128:(i + 1) * 128],
                                            in_=xcv[i][:, c0:c0 + Q],
                                            identity=ident_bf)
                    xtok = scan.tile([128, D_INNER], F16, tag="xtok")
                    for j in range(4):
                        nc.scalar.copy(out=xtok[:, j * 512:(j + 1) * 512],
                                       in_=xt_ps[:, j * 512:(j + 1) * 512])
                    xd = scan.tile([128, D_INNER], F16, tag="xd")
                    nc.vector.tensor_tensor(
                        out=xd.rearrange("p (h q) -> p h q", h=NHEADS),
                        in0=xtok.rearrange("p (h q) -> p h q", h=NHEADS),
                        in1=dtT[:, :, None].broadcast_to([128, NHEADS, HEADDIM]),
                        op=AluOpType.mult)
                    xw = scan.tile([128, D_INNER], F16, tag="xw")
                    nc.gpsimd.tensor_tensor(
                        out=xw.rearrange("p (h q) -> p h q", h=NHEADS),
                        in0=xtok.rearrange("p (h q) -> p h q", h=NHEADS),
                        in1=wT[:, :, None].broadcast_to([128, NHEADS, HEADDIM]),
                        op=AluOpType.mult)
                    s_scaled = scan.tile([D_STATE, D_INNER], F16, tag="s_scaled")
                    nc.gpsimd.tensor_tensor(
                        out=s_scaled.rearrange("p (h q) -> p h q", h=NHEADS),
                        in0=S_bf[sidx].rearrange("p (h q) -> p h q", h=NHEADS),
                        in1=RexpB[:, :, None].broadcast_to([128, NHEADS, HEADDIM]),
                        op=AluOpType.mult)

                    # inter-chunk contribution + D-skip
                    yi_ps = ps_big.tile([Q, D_INNER], F32, tag="ybig")
                    for j in range(4):
                        nc.tensor.matmul(out=yi_ps[:, j * 512:(j + 1) * 512],
                                         lhsT=CfmC,
                                         rhs=S_bf[sidx][:, j * 512:(j + 1) * 512],
                                         start=True, stop=True)
                    yacc = scan.tile([Q, D_INNER], F32, tag="yacc")
                    nc.vector.tensor_tensor(
                        out=yacc.rearrange("p (h q) -> p h q", h=NHEADS),
                        in0=yi_ps.rearrange("p (h q) -> p h q", h=NHEADS),
                        in1=etok[:, :, None].broadcast_to([128, NHEADS, HEADDIM]),
                        op=AluOpType.mult)
                    dskip = scan.tile([Q, D_INNER], F16, tag="dskip")
                    nc.gpsimd.tensor_tensor(
                        out=dskip.rearrange("p (h q) -> p h q", h=NHEADS),
                        in0=xtok.rearrange("p (h q) -> p h q", h=NHEADS),
                        in1=d_bc[:, :, None].broadcast_to([128, NHEADS, HEADDIM]),
                        op=AluOpType.mult)
                    acc = scan.tile([Q, D_INNER], F32, tag="t3")
                    nc.gpsimd.tensor_tensor(out=acc, in0=yacc, in1=dskip,
                                            op=AluOpType.add)

                    # intra-chunk attention (per head)
                    ya_ps = ps_big.tile([Q, D_INNER], F32, tag="ybig")
                    for h in range(NHEADS):
                        nc.tensor.matmul(
                            out=ya_ps[:, h * HEADDIM:(h + 1) * HEADDIM],
                            lhsT=m_bf[:, h * Q:(h + 1) * Q],
                            rhs=xd[:, h * HEADDIM:(h + 1) * HEADDIM],
                            start=True, stop=True)
                    t3 = scan.tile([Q, D_INNER], F32, tag="yacc")
                    nc.vector.tensor_tensor(out=t3, in0=ya_ps, in1=acc,
                                            op=AluOpType.add)

                    # state update
                    bt_ps = ps_sm.tile([Q, D_STATE], F16, tag="psA")
                    nc.tensor.transpose(out=bt_ps, in_=BfmC, identity=ident_bf)
                    btok = scs.tile([Q, D_STATE], F16, tag="btok")
                    nc.scalar.copy(out=btok, in_=bt_ps)
                    s_ps = ps_big.tile([D_STATE, D_INNER], F32, tag="ybig")
                    for j in range(4):
                        nc.tensor.matmul(out=s_ps[:, j * 512:(j + 1) * 512],
                                         lhsT=btok,
                                         rhs=xw[:, j * 512:(j + 1) * 512],
                                         start=True, stop=True)
                    nc.vector.tensor_tensor(out=S_bf[1 - sidx], in0=s_ps,
                                            in1=s_scaled, op=AluOpType.add)
                    sidx = 1 - sidx

                    # gate + rmsnorm
                    zchunk = swork.tile([Q, D_INNER], F16, tag="zchunk")
                    nc.sync.dma_start(out=zchunk,
                                      in_=z_dram[l0 + c0: l0 + c0 + Q, :])
                    yg = scan.tile([Q, D_INNER], F32, tag="t3")
                    nc.vector.tensor_tensor(out=yg, in0=t3, in1=zchunk,
                                            op=AluOpType.mult)
                    sq2 = swork.tile([Q, D_INNER], F16, tag="sq2")
                    ssq2 = scs.tile([Q, 1], F32, tag="ssq2")
                    nc.scalar.activation(out=sq2, in_=yg, func=AF.Square,
                                         accum_out=ssq2)
                    lny = scs.tile([Q, 1], F32, tag="lny")
                    nc.scalar.activation(out=lny, in_=ssq2, func=AF.Ln,
                                         bias=eps2, scale=1.0 / D_INNER)
                    rstd2 = scs.tile([Q, 1], F32, tag="rstd2")
                    nc.scalar.activation(out=rstd2, in_=lny, func=AF.Exp,
                                         scale=-0.5)
                    yn = scan.tile([Q, D_INNER], F16, tag="yn")
                    nc.vector.tensor_scalar(out=yn, in0=yg, scalar1=rstd2,
                                            scalar2=None, op0=AluOpType.mult)

                    # transpose y_n to feature-major into the l-group buffer
                    yt_ps = ps_bf.tile([128, D_INNER], F16, tag="xtps")
                    for i in range(16):
                        nc.tensor.transpose(out=yt_ps[:, i * 128:(i + 1) * 128],
                                            in_=yn[:, i * 128:(i + 1) * 128],
                                            identity=ident_bf)
                    gcol = (gk % 4) * 128
                    for i in range(16):
                        nc.scalar.copy(out=ynbuf[i][:, gcol:gcol + 128],
                                       in_=yt_ps[:, i * 128:(i + 1) * 128])

                    # every 4 chunks: out_proj + final projection + store
                    if gk % 4 == 3:
                        g = gk // 4
                        out1 = [scan.tile([128, 512], F16, tag=f"out1{m}", name=f"out1_{gk}_{m}")
                                for m in range(8)]
                        for m in range(8):
                            op_ps = ps_sm.tile([128, 512], F32, tag="psA")
                            for kt in range(16):
                                wo = wpool.tile([128, 128], F16, tag="wo")
                                nc.sync.dma_start(
                                    out=wo,
                                    in_=wout[kt * 128:(kt + 1) * 128,
                                             m * 128:(m + 1) * 128])
                                nc.tensor.matmul(out=op_ps, lhsT=wo,
                                                 rhs=ynbuf[kt],
                                                 start=(kt == 0), stop=(kt == 15))
                            nc.scalar.copy(out=out1[m], in_=op_ps)
                        for lt2 in range(4):
                            for j in range(2):
                                o_ps = ps_sm.tile([128, 512], F32, tag="psA")
                                for m in range(8):
                                    nc.tensor.matmul(
                                        out=o_ps,
                                        lhsT=out1[m][:, lt2 * 128:(lt2 + 1) * 128],
                                        rhs=wp_sb[m][:, j * 512:(j + 1) * 512],
                                        start=(m == 0), stop=(m == 7))
                                o_sb = swork.tile([128, 512], F32, tag="osb")
                                nc.scalar.copy(out=o_sb, in_=o_ps)
                                nc.sync.dma_start(
                                    out=o[g * 512 + lt2 * 128:
                                          g * 512 + (lt2 + 1) * 128,
                                          j * 512:(j + 1) * 512],
                                    in_=o_sb)


# ---------------------------------------------------------------------------
# Host wrapper
# ---------------------------------------------------------------------------
_NC_CACHE = None


def _get_program():
    global _NC_CACHE
    if _NC_CACHE is None:
        _NC_CACHE = build_program()
    return _NC_CACHE


def _per_core_inputs(inputs):
    x = np.asarray(inputs["x"], np.float32)
    norm_w = np.asarray(inputs["norm_w"], np.float32)
    tri = np.triu(np.ones((Q, Q), np.float32)).astype(np.float16)
    ident = np.eye(128, dtype=np.float32)
    in_maps = []
    shared = {}
    for dirn, pre in enumerate(("fwd_", "bwd_")):
        in_W = np.asarray(inputs[pre + "in_W"], np.float32) * norm_w[None, :]
        wxbc = np.ascontiguousarray(in_W[D_INNER:].T)
        wz = np.ascontiguousarray(in_W[:D_INNER].T)
        out_W = (np.asarray(inputs[pre + "out_W"], np.float32)
                 * np.asarray(inputs[pre + "gnorm_w"], np.float32)[None, :])
        wout = np.ascontiguousarray(out_W.T).astype(np.float16)
        pW = np.asarray(inputs["proj_W"], np.float32)
        wp_half = np.ascontiguousarray(
            pW[:, dirn * D_MODEL:(dirn + 1) * D_MODEL].T).astype(np.float16)
        shared[dirn] = dict(
            wxbc=wxbc, wz=wz, wout=wout, wp=wp_half,
            convw=np.asarray(inputs[pre + "conv_w"], np.float32),
            convb=np.asarray(inputs[pre + "conv_b"], np.float32),
            dtb=np.asarray(inputs[pre + "dt_bias"], np.float32),
            alog=np.asarray(inputs[pre + "A_log"], np.float32),
            dvec=np.asarray(inputs[pre + "D"], np.float32),
            tri=tri, identw=ident,
        )
    for c in range(8):
        b, dirn = c % 4, c // 4
        xb = x[b] if dirn == 0 else x[b, ::-1]
        m = dict(shared[dirn])
        m["xT"] = np.ascontiguousarray(xb.T)
        in_maps.append(m)
    return in_maps


def run_cores(inputs, trace=False):
    nc = _get_program()
    in_maps = _per_core_inputs(inputs)
    return run_bass_kernel_spmd(nc, in_maps, list(range(8)), trace=trace)


def kernel(**inputs):
    res = run_cores(inputs)
    x = np.asarray(inputs["x"], np.float32)
    proj_b = np.asarray(inputs["proj_b"], np.float32)
    out = np.empty_like(x)
    for b in range(4):
        out[b] = (x[b] + proj_b
                  + res.results[b]["o"]
                  + res.results[4 + b]["o"][::-1])
    return out
